# revision 2
# baseline (speedup 1.0000x reference)
"""Trainium2 Bass kernel for nn_CombineConcat (pairwise broadcast+concat).

reference semantics (per batch b):
  out[b, i*N + j, 0:D]   = x1[b, i, :]
  out[b, i*N + j, D:2*D] = x2[b, j, :]

Shapes (hardcoded): x1, x2 = [16, 128, 256] f32 -> out = [16, 16384, 512] f32.

Strategy: data-parallel over batch, 2 batches per core on 8 cores. Output
write-bound (64 MB/core). j-major SBUF layout: each ring slot is
[128, 8*512] f32 where partition p holds 8 consecutive output rows
(16 KB contiguous per partition) of a 1024-row group g:
  row g*1024 + 8p + r  =  [x1[8g + p//16] | x2[8*(p%16) + r]]
16 KB-contiguous descriptors sustain ~370 GB/s/core vs ~310 GB/s for the
2 KB row-major layout, which is the whole win: pure-write floor 173 us vs
211 us. Both halves are materialized by one-hot selector matmuls on the
otherwise-idle PE (x1: K=24 replicates row 8g+u to partition group u; x2:
K=48 replicates x2 across the 8 partition groups once per batch), fanned
into slots by DVE/ACT broadcast-read copies. Inputs are pre-split on the
host into 3 bf16 terms (hi/lo1/lo2) stacked on K, so one matmul sums them
with every partial sum exactly representable -> bit-exact f32 output.
Slot-0's dependencies ride in the first input DMA to shorten the ramp;
the last group's DMA is split across both HWDGE queues to halve the tail.
"""

import numpy as np

_B, _N, _D = 16, 128, 256
_NCORES = 8
_BPC = _B // _NCORES  # batches per core
_ROWS = 8  # output rows per partition per slot
_GRP = _N * _ROWS  # dram rows per output dma (1024)
_NGRP = _N * _N // _GRP  # groups per batch (16)
_NSLOTS = 6

_NC_CACHE = {}


def _build_nc():
    import concourse.bacc as bacc
    import concourse.mybir as mybir
    from concourse.tile import TileContext
    from concourse.bass import MemorySpace

    f32 = mybir.dt.float32
    bf16 = mybir.dt.bfloat16
    bpc, n, d = _BPC, _N, _D
    W = _ROWS * 2 * d  # 4096 f32 per partition per slot
    nq = n // _ROWS  # 16 partition-groups / x2 rows per group

    nc = bacc.Bacc("TRN2", target_bir_lowering=False, enable_partition_id=False)
    # host-prearranged inputs (see _run). x1/x2 are split into 3 exact bf16
    # terms (hi/lo1/lo2) stacked on the matmul K (partition) dim, so one
    # matmul sums all three terms (every partial sum is exactly
    # representable, so the result is bit-exact f32):
    #   x1all[b, 8j+u, g*256+c] = term_j(x1[b, 8g+u, c])      K=24
    #   x2all[b, 16j+q, r*256+c] = term_j(x2[b, 8q+r, c])     K=48
    # selall cols 0:128 = sel2_3 [48,128]: [16j+q, p]=1 iff p%16==q
    #        cols 128:256 rows 0:24 = sel1_3 [24,128]: [8j+u, p]=1 iff p//16==u
    x1all = nc.dram_tensor("x1all", [bpc, 3 * _ROWS, _NGRP * d], bf16, kind="ExternalInput")
    x2all = nc.dram_tensor("x2all", [bpc, 3 * nq, _ROWS * d], bf16, kind="ExternalInput")
    selall = nc.dram_tensor("selall", [3 * nq, 2 * n + d + _ROWS * d], bf16, kind="ExternalInput")
    out = nc.dram_tensor("out", [bpc, n * n, 2 * d], f32, kind="ExternalOutput")

    with TileContext(nc) as tc:
        with (
            tc.tile_pool(name="io", bufs=1) as iop,
            tc.tile_pool(name="ring", bufs=1) as rp,
            tc.tile_pool(name="ps", bufs=1, space=MemorySpace.PSUM) as pp,
        ):
            selsb = iop.tile([3 * nq, 2 * n + d + _ROWS * d], bf16, name="selsb", tag="selsb")
            sel2ap = selsb[:, 0:n]
            sel1ap = selsb[0 : 3 * _ROWS, n : 2 * n]
            # batch-0 g=0 x1 rhs and full x2 rhs ride in the first DMA so the
            # slot-0 fill chain waits on one early load only
            x1g0ap = selsb[0 : 3 * _ROWS, 2 * n : 2 * n + d]
            x2b0ap = selsb[:, 2 * n + d : 2 * n + d + _ROWS * d]
            x1t = [
                iop.tile([3 * _ROWS, _NGRP * d], bf16, name=f"x1t_{b}", tag=f"x1t_{b}")
                for b in range(bpc)
            ]
            x2t = [
                iop.tile([3 * nq, _ROWS * d], bf16, name=f"x2t_{b}", tag=f"x2t_{b}")
                for b in range(bpc)
            ]

            def load_batch(b):
                # split each load across both queues for partition/engine
                # parallelism (few-partition DMAs are SBUF-port serial)
                h1 = _NGRP * d // 2
                nc.sync.dma_start(out=x1t[b][:, 0:h1], in_=x1all[b][:, 0:h1])
                nc.scalar.dma_start(out=x1t[b][:, h1:], in_=x1all[b][:, h1:])
                h2 = _ROWS * d // 2
                nc.scalar.dma_start(out=x2t[b][:, 0:h2], in_=x2all[b][:, 0:h2])
                nc.sync.dma_start(out=x2t[b][:, h2:], in_=x2all[b][:, h2:])

            nc.sync.dma_start(out=selsb[:], in_=selall[:, :])
            load_batch(0)
            load_batch(1)

            slots = [rp.tile([n, W], f32, name=f"s{k}", tag=f"s{k}") for k in range(_NSLOTS)]
            px2 = pp.tile([n, _ROWS * d], f32, name="px2", tag="px2")
            px1 = [pp.tile([n, 512], f32, name=f"px1_{k}", tag=f"px1_{k}") for k in range(4)]

            def x1_mm(b, g, k):
                p1 = px1[k % 4][:, 0:d]
                rhs = (
                    x1g0ap
                    if (b == 0 and g == 0)
                    else x1t[b][:, g * d : (g + 1) * d]
                )
                nc.tensor.matmul(p1, sel1ap, rhs, start=True, stop=True)
                return p1

            di = 0
            for b in range(bpc):
                sv0 = slots[0][:].rearrange("p (r h c) -> p r h c", r=_ROWS, h=2)
                # slot-0 x1 matmul first so its DVE fanout overlaps px2 mms
                p1_first = x1_mm(b, 0, 0)
                nc.vector.tensor_copy(
                    out=sv0[:, :, 0, :],
                    in_=p1_first[:, None, :].broadcast_to((n, _ROWS, d)),
                )
                # replicate x2[b] across partition groups on the PE:
                # px2[p, r*256+c] = x2[b, 8*(p%16)+r, c]; chunked so slot-0
                # x2 fill starts after the first matmul
                for j in range(4):
                    cs = slice(j * 512, (j + 1) * 512)
                    x2rhs = x2b0ap[:, cs] if b == 0 else x2t[b][:, cs]
                    nc.tensor.matmul(px2[:, cs], sel2ap, x2rhs, start=True, stop=True)
                    nc.vector.tensor_copy(
                        out=sv0[:, 2 * j : 2 * j + 2, 1, :],
                        in_=px2[:, cs].rearrange("p (r c) -> p r c", r=2),
                    )
                for g in range(_NGRP):
                    k = g % _NSLOTS
                    sv = slots[k][:].rearrange("p (r h c) -> p r h c", r=_ROWS, h=2)
                    if g > 0:
                        if g < _NSLOTS:  # x2 half: once per slot per batch
                            nc.vector.tensor_copy(
                                out=sv[:, :, 1, :],
                                in_=px2[:].rearrange("p (r c) -> p r c", r=_ROWS),
                            )
                        p1 = x1_mm(b, g, g)
                        ceng = nc.vector if g % 2 == 0 else nc.scalar
                        cop = ceng.tensor_copy if g % 2 == 0 else ceng.copy
                        cop(
                            out=sv[:, :, 0, :],
                            in_=p1[:, None, :].broadcast_to((n, _ROWS, d)),
                        )
                    dst = out[b][g * _GRP : (g + 1) * _GRP, :].rearrange(
                        "(p r) c -> p (r c)", p=n
                    )
                    if b == bpc - 1 and g == _NGRP - 1:
                        nc.sync.dma_start(out=dst[0:64, :], in_=slots[k][0:64, :])
                        nc.scalar.dma_start(out=dst[64:n, :], in_=slots[k][64:n, :])
                    else:
                        eng = nc.sync if di % 2 == 0 else nc.scalar
                        eng.dma_start(out=dst, in_=slots[k][:])
                    di += 1
    nc.finalize()
    return nc


def _get_nc():
    if "nc" not in _NC_CACHE:
        _NC_CACHE["nc"] = _build_nc()
    return _NC_CACHE["nc"]


def _split_bf16_3(x):
    """x (f32) == hi + lo1 + lo2 exactly, each exactly bf16-representable."""
    xu = x.view(np.uint32)
    hi = (xu & np.uint32(0xFFFF0000)).view(np.float32)
    r1 = x - hi
    r1u = r1.view(np.uint32)
    lo1 = (r1u & np.uint32(0xFFFF0000)).view(np.float32)
    lo2 = r1 - lo1
    import ml_dtypes

    return (
        hi.astype(ml_dtypes.bfloat16),
        lo1.astype(ml_dtypes.bfloat16),
        lo2.astype(ml_dtypes.bfloat16),
    )


def _run(x1, x2, trace=False):
    """Run the kernel on 8 cores; returns (output, BassKernelResults)."""
    from concourse.bass_utils import run_bass_kernel_spmd

    nc = _get_nc()
    x1 = np.ascontiguousarray(np.asarray(x1, dtype=np.float32))
    x2 = np.ascontiguousarray(np.asarray(x2, dtype=np.float32))
    import ml_dtypes

    nq = _N // _ROWS
    sel1 = (
        np.arange(_N)[None, :] // 16 == np.arange(_ROWS)[:, None]
    ).astype(np.float32)
    sel2 = (
        np.arange(_N)[None, :] % 16 == np.arange(nq)[:, None]
    ).astype(np.float32)
    selbase = np.zeros((3 * nq, 2 * _N), np.float32)
    selbase[:, 0:_N] = np.tile(sel2, (3, 1))
    selbase[0 : 3 * _ROWS, _N : 2 * _N] = np.tile(sel1, (3, 1))
    selbase = selbase.astype(ml_dtypes.bfloat16)
    in_maps = []
    for c in range(_NCORES):
        x1s = x1[c * _BPC : (c + 1) * _BPC]
        x2s = x2[c * _BPC : (c + 1) * _BPC]
        # x1g[b, u, g*256+c] = x1[b, 8g+u, c]
        x1g = np.ascontiguousarray(
            x1s.reshape(_BPC, _NGRP, _ROWS, _D).transpose(0, 2, 1, 3)
        ).reshape(_BPC, _ROWS, _NGRP * _D)
        x1all = np.concatenate(_split_bf16_3(x1g), axis=1)  # [bpc, 24, 4096]
        x2all = np.concatenate(
            _split_bf16_3(x2s.reshape(_BPC, nq, _ROWS * _D)), axis=1
        )  # [bpc, 48, 2048]
        x1g0 = np.zeros((3 * nq, _D), ml_dtypes.bfloat16)
        x1g0[0 : 3 * _ROWS] = x1all[0][:, 0:_D]
        selall = np.concatenate([selbase, x1g0, x2all[0]], axis=1)
        in_maps.append(
            {
                "x1all": np.ascontiguousarray(x1all),
                "x2all": np.ascontiguousarray(x2all),
                "selall": np.ascontiguousarray(selall),
            }
        )
    res = run_bass_kernel_spmd(
        nc, in_maps, core_ids=list(range(_NCORES)), trace=trace
    )
    out = np.concatenate([r["out"] for r in res.results], axis=0)
    return out, res


def kernel(x1, x2):
    out, _ = _run(x1, x2, trace=False)
    return out


# revision 3
# speedup vs baseline: 1.1218x; 1.1218x over previous
"""Trainium2 Bass kernel for nn_CombineConcat (pairwise broadcast+concat).

reference semantics (per batch b):
  out[b, i*N + j, 0:D]   = x1[b, i, :]
  out[b, i*N + j, D:2*D] = x2[b, j, :]

Shapes (hardcoded): x1, x2 = [16, 128, 256] f32 -> out = [16, 16384, 512] f32.

Strategy: data-parallel over batch, 2 batches per core on 8 cores. Output
write-bound (64 MB/core). j-major SBUF layout: each ring slot is
[128, 8*512] f32 where partition p holds 8 consecutive output rows
(16 KB contiguous per partition) of a 1024-row group g:
  row g*1024 + 8p + r  =  [x1[8g + p//16] | x2[8*(p%16) + r]]
16 KB-contiguous descriptors sustain ~370 GB/s/core vs ~310 GB/s for the
2 KB row-major layout (pure-write floor 173 us vs 211 us) -- that layout
switch is the main win. Both halves are materialized by one-hot selector
matmuls on the otherwise-idle PE (x1: K=24 replicates row 8g+u to
partition group u per 2 MB output group; x2: K=48 replicates x2[b] across
the 8 partition groups once per batch), fanned into slots by DVE/ACT
broadcast-read copies. Inputs are pre-split on the host into 3 bf16 terms
(hi/lo1/lo2) stacked on K so one matmul sums them; every partial sum is
exactly representable, so the output is bit-exact f32. Slot-0's
dependencies ride in the first input DMA to shorten the ramp; the slot
mapping rotates by 4 between batches so batch-1 x2 refills overlap
batch-0's tail; the last two groups' DMAs are split across both HWDGE
queues to flatten the drain.
"""

import numpy as np

_B, _N, _D = 16, 128, 256
_NCORES = 8
_BPC = _B // _NCORES  # batches per core
_ROWS = 8  # output rows per partition per slot
_GRP = _N * _ROWS  # dram rows per output dma (1024)
_NGRP = _N * _N // _GRP  # groups per batch (16)
_NSLOTS = 6

_NC_CACHE = {}


def _build_nc():
    import concourse.bacc as bacc
    import concourse.mybir as mybir
    from concourse.tile import TileContext
    from concourse.bass import MemorySpace

    f32 = mybir.dt.float32
    bf16 = mybir.dt.bfloat16
    bpc, n, d = _BPC, _N, _D
    W = _ROWS * 2 * d  # 4096 f32 per partition per slot
    nq = n // _ROWS  # 16 partition-groups / x2 rows per group

    nc = bacc.Bacc("TRN2", target_bir_lowering=False, enable_partition_id=False)
    # host-prearranged inputs (see _run). x1/x2 are split into 3 exact bf16
    # terms (hi/lo1/lo2) stacked on the matmul K (partition) dim, so one
    # matmul sums all three terms (every partial sum is exactly
    # representable, so the result is bit-exact f32):
    #   x1all[b, 8j+u, g*256+c] = term_j(x1[b, 8g+u, c])      K=24
    #   x2all[b, 16j+q, r*256+c] = term_j(x2[b, 8q+r, c])     K=48
    # selall cols 0:128 = sel2_3 [48,128]: [16j+q, p]=1 iff p%16==q
    #        cols 128:256 rows 0:24 = sel1_3 [24,128]: [8j+u, p]=1 iff p//16==u
    x1all = nc.dram_tensor("x1all", [bpc, 3 * _ROWS, _NGRP * d], bf16, kind="ExternalInput")
    x2all = nc.dram_tensor("x2all", [bpc, 3 * nq, _ROWS * d], bf16, kind="ExternalInput")
    selall = nc.dram_tensor("selall", [3 * nq, 2 * n + d + _ROWS * d], bf16, kind="ExternalInput")
    out = nc.dram_tensor("out", [bpc, n * n, 2 * d], f32, kind="ExternalOutput")

    with TileContext(nc) as tc:
        with (
            tc.tile_pool(name="io", bufs=1) as iop,
            tc.tile_pool(name="ring", bufs=1) as rp,
            tc.tile_pool(name="ps", bufs=1, space=MemorySpace.PSUM) as pp,
        ):
            selsb = iop.tile([3 * nq, 2 * n + d + _ROWS * d], bf16, name="selsb", tag="selsb")
            sel2ap = selsb[:, 0:n]
            sel1ap = selsb[0 : 3 * _ROWS, n : 2 * n]
            # batch-0 g=0 x1 rhs and full x2 rhs ride in the first DMA so the
            # slot-0 fill chain waits on one early load only
            x1g0ap = selsb[0 : 3 * _ROWS, 2 * n : 2 * n + d]
            x2b0ap = selsb[:, 2 * n + d : 2 * n + d + _ROWS * d]
            x1t = [
                iop.tile([3 * _ROWS, _NGRP * d], bf16, name=f"x1t_{b}", tag=f"x1t_{b}")
                for b in range(bpc)
            ]
            x2t = [
                iop.tile([3 * nq, _ROWS * d], bf16, name=f"x2t_{b}", tag=f"x2t_{b}")
                for b in range(bpc)
            ]

            def load_batch(b):
                # split each load across both queues for partition/engine
                # parallelism (few-partition DMAs are SBUF-port serial)
                h1 = _NGRP * d // 2
                nc.sync.dma_start(out=x1t[b][:, 0:h1], in_=x1all[b][:, 0:h1])
                nc.scalar.dma_start(out=x1t[b][:, h1:], in_=x1all[b][:, h1:])
                h2 = _ROWS * d // 2
                nc.scalar.dma_start(out=x2t[b][:, 0:h2], in_=x2all[b][:, 0:h2])
                nc.sync.dma_start(out=x2t[b][:, h2:], in_=x2all[b][:, h2:])

            nc.sync.dma_start(out=selsb[:], in_=selall[:, :])
            load_batch(0)
            load_batch(1)

            slots = [rp.tile([n, W], f32, name=f"s{k}", tag=f"s{k}") for k in range(_NSLOTS)]
            px2 = pp.tile([n, _ROWS * d], f32, name="px2", tag="px2")
            px1 = [pp.tile([n, 512], f32, name=f"px1_{k}", tag=f"px1_{k}") for k in range(4)]

            def x1_mm(b, g, k):
                p1 = px1[k % 4][:, 0:d]
                rhs = (
                    x1g0ap
                    if (b == 0 and g == 0)
                    else x1t[b][:, g * d : (g + 1) * d]
                )
                nc.tensor.matmul(p1, sel1ap, rhs, start=True, stop=True)
                return p1

            di = 0
            for b in range(bpc):
                k0 = (4 * b) % _NSLOTS
                sv0 = slots[k0][:].rearrange("p (r h c) -> p r h c", r=_ROWS, h=2)
                # first-group x1 matmul first so its DVE fanout overlaps px2 mms
                p1_first = x1_mm(b, 0, k0)
                nc.vector.tensor_copy(
                    out=sv0[:, :, 0, :],
                    in_=p1_first[:, None, :].broadcast_to((n, _ROWS, d)),
                )
                # replicate x2[b] across partition groups on the PE:
                # px2[p, r*256+c] = x2[b, 8*(p%16)+r, c]; chunked so slot-0
                # x2 fill starts after the first matmul
                for j in range(4):
                    cs = slice(j * 512, (j + 1) * 512)
                    x2rhs = x2b0ap[:, cs] if b == 0 else x2t[b][:, cs]
                    nc.tensor.matmul(px2[:, cs], sel2ap, x2rhs, start=True, stop=True)
                    nc.vector.tensor_copy(
                        out=sv0[:, 2 * j : 2 * j + 2, 1, :],
                        in_=px2[:, cs].rearrange("p (r c) -> p r c", r=2),
                    )
                for g in range(_NGRP):
                    # rotate slot mapping per batch so batch-1's first slots
                    # are the ones batch-0 freed earliest (refills overlap
                    # batch-0's tail instead of stalling at the boundary)
                    k = (g + 4 * b) % _NSLOTS
                    sv = slots[k][:].rearrange("p (r h c) -> p r h c", r=_ROWS, h=2)
                    if g > 0:
                        if g < _NSLOTS:  # x2 half: once per slot per batch
                            nc.vector.tensor_copy(
                                out=sv[:, :, 1, :],
                                in_=px2[:].rearrange("p (r c) -> p r c", r=_ROWS),
                            )
                        p1 = x1_mm(b, g, g)
                        ceng = nc.vector if g % 2 == 0 else nc.scalar
                        cop = ceng.tensor_copy if g % 2 == 0 else ceng.copy
                        cop(
                            out=sv[:, :, 0, :],
                            in_=p1[:, None, :].broadcast_to((n, _ROWS, d)),
                        )
                    dst = out[b][g * _GRP : (g + 1) * _GRP, :].rearrange(
                        "(p r) c -> p (r c)", p=n
                    )
                    if b == bpc - 1 and g >= _NGRP - 2:
                        nc.sync.dma_start(out=dst[0:64, :], in_=slots[k][0:64, :])
                        nc.scalar.dma_start(out=dst[64:n, :], in_=slots[k][64:n, :])
                    else:
                        eng = nc.sync if di % 2 == 0 else nc.scalar
                        eng.dma_start(out=dst, in_=slots[k][:])
                    di += 1
    nc.finalize()
    return nc


def _get_nc():
    if "nc" not in _NC_CACHE:
        _NC_CACHE["nc"] = _build_nc()
    return _NC_CACHE["nc"]


def _split_bf16_3(x):
    """x (f32) == hi + lo1 + lo2 exactly, each exactly bf16-representable."""
    xu = x.view(np.uint32)
    hi = (xu & np.uint32(0xFFFF0000)).view(np.float32)
    r1 = x - hi
    r1u = r1.view(np.uint32)
    lo1 = (r1u & np.uint32(0xFFFF0000)).view(np.float32)
    lo2 = r1 - lo1
    import ml_dtypes

    return (
        hi.astype(ml_dtypes.bfloat16),
        lo1.astype(ml_dtypes.bfloat16),
        lo2.astype(ml_dtypes.bfloat16),
    )


def _run(x1, x2, trace=False):
    """Run the kernel on 8 cores; returns (output, BassKernelResults)."""
    from concourse.bass_utils import run_bass_kernel_spmd

    nc = _get_nc()
    x1 = np.ascontiguousarray(np.asarray(x1, dtype=np.float32))
    x2 = np.ascontiguousarray(np.asarray(x2, dtype=np.float32))
    import ml_dtypes

    nq = _N // _ROWS
    sel1 = (
        np.arange(_N)[None, :] // 16 == np.arange(_ROWS)[:, None]
    ).astype(np.float32)
    sel2 = (
        np.arange(_N)[None, :] % 16 == np.arange(nq)[:, None]
    ).astype(np.float32)
    selbase = np.zeros((3 * nq, 2 * _N), np.float32)
    selbase[:, 0:_N] = np.tile(sel2, (3, 1))
    selbase[0 : 3 * _ROWS, _N : 2 * _N] = np.tile(sel1, (3, 1))
    selbase = selbase.astype(ml_dtypes.bfloat16)
    in_maps = []
    for c in range(_NCORES):
        x1s = x1[c * _BPC : (c + 1) * _BPC]
        x2s = x2[c * _BPC : (c + 1) * _BPC]
        # x1g[b, u, g*256+c] = x1[b, 8g+u, c]
        x1g = np.ascontiguousarray(
            x1s.reshape(_BPC, _NGRP, _ROWS, _D).transpose(0, 2, 1, 3)
        ).reshape(_BPC, _ROWS, _NGRP * _D)
        x1all = np.concatenate(_split_bf16_3(x1g), axis=1)  # [bpc, 24, 4096]
        x2all = np.concatenate(
            _split_bf16_3(x2s.reshape(_BPC, nq, _ROWS * _D)), axis=1
        )  # [bpc, 48, 2048]
        x1g0 = np.zeros((3 * nq, _D), ml_dtypes.bfloat16)
        x1g0[0 : 3 * _ROWS] = x1all[0][:, 0:_D]
        selall = np.concatenate([selbase, x1g0, x2all[0]], axis=1)
        in_maps.append(
            {
                "x1all": np.ascontiguousarray(x1all),
                "x2all": np.ascontiguousarray(x2all),
                "selall": np.ascontiguousarray(selall),
            }
        )
    res = run_bass_kernel_spmd(
        nc, in_maps, core_ids=list(range(_NCORES)), trace=trace
    )
    out = np.concatenate([r["out"] for r in res.results], axis=0)
    return out, res


def kernel(x1, x2):
    out, _ = _run(x1, x2, trace=False)
    return out
